# revision 1
# baseline (speedup 1.0000x reference)
"""CAFE-interpolation kernel for 8 Trainium2 NeuronCores.

Strategy: shard the T axis (1024 = 8 x 128) across cores. Every core holds a
T-slice of ALL 128 samples, so the sr[partner_idx] gather is core-local.

Math: with mask_b = (im_b > thr_b) in {0,1}^D and c_b = is_dominant_b*(1-m_b):

  out[b] = x[b] + c_b * ( mask[p_b] . x[p_b] - mask[b] . x[b] )
         = x[b] + c_b * ((P - I) @ (mask . x))[b]

so the whole mixup collapses into one constant-permutation matmul over the
sample axis plus elementwise ops:

  stage 1: im_partial[b, d] = sum_{t in slice} grad[b,t,d]*x[b,t,d]
           All on DVE: elementwise mul + strided free-axis reduce over t
           (samples live on partitions), accumulate across t-groups,
           scale by 1/1024 at the end.
  AllReduce im_partial [128, 512] across the 8 cores (~256 KB).
  stage 2: exact 52nd/53rd largest of each im row: iterative max-extraction
           with fused mask-out+reduce (tensor_scalar + tensor_tensor_reduce),
           thr = v459 + 0.9*(v460-v459) exactly like jnp.quantile,
           mask = im > thr; cvec = is_dominant*(1-mixup).
  stage 3: per t-pair: xm = x[:,t,:] * mask        (DVE / GpSimd alternating)
                       q  = (P-I)^T @ xm           (PE, constant weights)
                       out = (q * cvec) + x[:,t,:] (fused scalar_tensor_tensor)

The same program works for every (partner_idx, is_dominant): the metadata
enters only through the pmi/dom input tensors, so it compiles once per
process.
"""

import os
import numpy as np

B, T, D = 128, 1024, 512
N_CORES = 8
T_LOC = T // N_CORES  # 128
KTOP = 53  # need the 52nd and 53rd largest of each 512-row
TG1 = 8  # t-steps per stage-1 group
TG3 = 2  # t-steps per stage-3 group

_CACHE: dict = {}
LAST_RESULT = None


def _build():
    import concourse.mybir as mybir
    import concourse.tile as tile
    from concourse import bacc

    f32 = mybir.dt.float32
    Alu = mybir.AluOpType
    AX = mybir.AxisListType

    _dbg = os.environ.get("KBUILD_DEBUG") == "1"

    nc = bacc.Bacc(
        "TRN2", target_bir_lowering=False, debug=False, num_devices=N_CORES
    )
    x_sl = nc.dram_tensor("x_sl", [B, T_LOC, D], f32, kind="ExternalInput")
    g_sl = nc.dram_tensor("g_sl", [B, T_LOC, D], f32, kind="ExternalInput")
    m_in = nc.dram_tensor("m_in", [B, 1], f32, kind="ExternalInput")
    dom_in = nc.dram_tensor("dom_in", [B, 1], f32, kind="ExternalInput")
    pmi_in = nc.dram_tensor("pmi_in", [B, B], f32, kind="ExternalInput")
    out_sl = nc.dram_tensor("out_sl", [B, T_LOC, D], f32, kind="ExternalOutput")
    if _dbg:
        dbg_im = nc.dram_tensor("dbg_im", [B, D], f32, kind="ExternalOutput")
        dbg_mask = nc.dram_tensor("dbg_mask", [B, D], f32, kind="ExternalOutput")

    with tile.TileContext(nc) as tc:
        with tc.tile_pool(name="persist", bufs=1) as pp:
            m_t = pp.tile([B, 1], f32)
            nc.sync.dma_start(m_t[:], m_in[:])
            dom_t = pp.tile([B, 1], f32)
            nc.sync.dma_start(dom_t[:], dom_in[:])
            pmi_t = pp.tile([B, B], f32)
            nc.sync.dma_start(pmi_t[:], pmi_in[:])
            im_all = pp.tile([B, D], f32)
            cur_a = pp.tile([B, D], f32)
            cur_b = pp.tile([B, D], f32)
            mv = pp.tile([B, 64], f32)
            mask = pp.tile([B, D], f32)
            cvec = pp.tile([B, 1], f32)
            imacc = pp.tile([B, D], f32)

            # ---- stage 1: im_partial = sum_t x*g on DVE ----
            with (
                tc.tile_pool(name="ld1", bufs=2) as ld1,
                tc.tile_pool(name="pr1", bufs=2) as pr1,
                tc.tile_pool(name="ccp", bufs=1, space="DRAM") as ccp,
            ):
                n_g1 = T_LOC // TG1
                for i in range(n_g1):
                    t0 = i * TG1
                    xt = ld1.tile([B, TG1, D], f32, tag="x1")
                    gt = ld1.tile([B, TG1, D], f32, tag="g1")
                    nc.sync.dma_start(xt[:], x_sl[:, t0 : t0 + TG1, :])
                    nc.sync.dma_start(gt[:], g_sl[:, t0 : t0 + TG1, :])
                    prod = pr1.tile([B, TG1, D], f32, tag="prod")
                    nc.vector.tensor_tensor(prod[:], xt[:], gt[:], op=Alu.mult)
                    # contiguous pairwise tree-sum over t (the strided-innermost
                    # tensor_reduce measures ~1.6x slower than dense adds)
                    f4 = pr1.tile([B, TG1 // 2, D], f32, tag="f4")
                    nc.vector.tensor_tensor(
                        f4[:], prod[:, 0 : TG1 // 2, :], prod[:, TG1 // 2 :, :],
                        op=Alu.add,
                    )
                    f2 = pr1.tile([B, TG1 // 4, D], f32, tag="f2")
                    nc.vector.tensor_tensor(
                        f2[:], f4[:, 0 : TG1 // 4, :], f4[:, TG1 // 4 :, :],
                        op=Alu.add,
                    )
                    if i == 0:
                        nc.vector.tensor_tensor(
                            imacc[:], f2[:, 0, :], f2[:, 1, :], op=Alu.add
                        )
                    else:
                        part = pr1.tile([B, D], f32, tag="part")
                        nc.vector.tensor_tensor(
                            part[:], f2[:, 0, :], f2[:, 1, :], op=Alu.add
                        )
                        nc.vector.tensor_tensor(
                            imacc[:], imacc[:], part[:], op=Alu.add
                        )
                # scale by 1/T (exact power of two)
                nc.vector.tensor_scalar(
                    imacc[:], imacc[:], scalar1=1.0 / T, scalar2=None, op0=Alu.mult
                )

                # ---- AllReduce the partial importance ----
                cc_in_t = ccp.tile([B, D], f32, name="cc_in_t")
                cc_out_t = ccp.tile([B, D], f32, name="cc_out_t")
                nc.gpsimd.dma_start(cc_in_t[:], imacc[:])
                nc.gpsimd.collective_compute(
                    "AllReduce",
                    Alu.add,
                    replica_groups=[list(range(N_CORES))],
                    ins=[cc_in_t.opt()],
                    outs=[cc_out_t.opt()],
                )
                nc.gpsimd.dma_start(im_all[:], cc_out_t[:])

            # ---- stage 2: exact top-52/53 values per row ----
            with (
                tc.tile_pool(name="sel", bufs=2) as selp,
                tc.tile_pool(name="psumw", bufs=1, space="PSUM") as psumw,
            ):
                # Iterative exact max-extraction. Removed elements become 0,
                # which is a safe sentinel because the top-53 of a 512-wide
                # zero-mean row are positive (P(not) ~ 1e-90 for randn data);
                # surviving values are untouched (exact order statistics).
                cur, nxt = im_all, cur_b
                nc.vector.reduce_max(mv[:, 0:1], cur[:], axis=AX.X)
                for k in range(1, KTOP):
                    # cur' = (cur < m_{k-1}) * cur ; mv[k] = max(cur')
                    nc.vector.scalar_tensor_tensor(
                        nxt[:],
                        cur[:],
                        mv[:, k - 1 : k],
                        cur[:],
                        op0=Alu.is_lt,
                        op1=Alu.mult,
                    )
                    nc.vector.reduce_max(mv[:, k : k + 1], nxt[:], axis=AX.X)
                    cur = nxt
                    nxt = cur_a if cur is cur_b else cur_b

                # PE warm-up during the selection window (junk results)
                qw = psumw.tile([B, D], f32)
                for _ in range(20):
                    nc.tensor.matmul(
                        qw[:], pmi_t[:], im_all[:], start=True, stop=True
                    )

                # thr = v459 + 0.9*(v460 - v459); v460 = mv[:,51], v459 = mv[:,52]
                dl = pp.tile([B, 1], f32)
                nc.vector.tensor_tensor(
                    dl[:], mv[:, 51:52], mv[:, 52:53], op=Alu.subtract
                )
                dl9 = pp.tile([B, 1], f32)
                nc.vector.tensor_scalar(
                    dl9[:], dl[:], scalar1=0.9, scalar2=None, op0=Alu.mult
                )
                thr_t = pp.tile([B, 1], f32)
                nc.vector.tensor_tensor(thr_t[:], mv[:, 52:53], dl9[:], op=Alu.add)

                mask_src = im_all
                nc.vector.tensor_scalar(
                    mask[:],
                    mask_src[:],
                    scalar1=thr_t[:, 0:1],
                    scalar2=None,
                    op0=Alu.is_gt,
                )

                # cvec = dom * (1 - m)
                om_t = pp.tile([B, 1], f32)
                nc.vector.tensor_scalar(
                    om_t[:],
                    m_t[:],
                    scalar1=-1.0,
                    scalar2=1.0,
                    op0=Alu.mult,
                    op1=Alu.add,
                )
                nc.vector.tensor_tensor(cvec[:], om_t[:], dom_t[:], op=Alu.mult)

                if _dbg:
                    nc.gpsimd.dma_start(dbg_im[:], im_all[:])
                    nc.gpsimd.dma_start(dbg_mask[:], mask[:])

            # ---- stage 3: out = x + c * ((P-I) @ (mask.x)) ----
            with (
                tc.tile_pool(name="x3", bufs=36) as x3p,
                tc.tile_pool(name="t3", bufs=4) as t3p,
                tc.tile_pool(name="psumq", bufs=3, space="PSUM") as psumq,
            ):
                for gi, t0 in enumerate(range(0, T_LOC, TG3)):
                    xt3 = x3p.tile([B, TG3, D], f32, tag="x3t")
                    nc.sync.dma_start(xt3[:], x_sl[:, t0 : t0 + TG3, :])
                    q = psumq.tile([B, TG3, D], f32, tag="q")
                    ot = t3p.tile([B, TG3, D], f32, tag="ot")
                    # one wide mask-multiply for the whole t-pair; mask is
                    # broadcast over t by a zero-stride middle AP dim
                    xm = t3p.tile([B, TG3, D], f32, tag="xm")
                    eng = nc.vector if gi % 2 == 0 else nc.gpsimd
                    for j in range(TG3):
                        eng.tensor_tensor(
                            xm[:, j, :], xt3[:, j, :], mask[:], op=Alu.mult
                        )
                    for j in range(TG3):
                        nc.tensor.matmul(
                            q[:, j, :], pmi_t[:], xm[:, j, :], start=True, stop=True
                        )
                    # out = (q * c) + x over the whole t-pair at once
                    nc.vector.scalar_tensor_tensor(
                        ot[:],
                        q[:],
                        cvec[:, 0:1],
                        xt3[:],
                        op0=Alu.mult,
                        op1=Alu.add,
                    )
                    nc.scalar.dma_start(out_sl[:, t0 : t0 + TG3, :], ot[:])
    nc.compile()
    return nc


def _build_copy():
    """All-non-dominant fast path: output == x."""
    import concourse.mybir as mybir
    import concourse.tile as tile
    from concourse import bacc

    f32 = mybir.dt.float32
    nc = bacc.Bacc(
        "TRN2", target_bir_lowering=False, debug=False, num_devices=N_CORES
    )
    x_sl = nc.dram_tensor("x_sl", [B, T_LOC, D], f32, kind="ExternalInput")
    nc.dram_tensor("g_sl", [B, T_LOC, D], f32, kind="ExternalInput")
    nc.dram_tensor("m_in", [B, 1], f32, kind="ExternalInput")
    nc.dram_tensor("dom_in", [B, 1], f32, kind="ExternalInput")
    nc.dram_tensor("pmi_in", [B, B], f32, kind="ExternalInput")
    out_sl = nc.dram_tensor("out_sl", [B, T_LOC, D], f32, kind="ExternalOutput")
    with tile.TileContext(nc):
        CG = 8
        for i, b0 in enumerate(range(0, B, CG)):
            eng = nc.sync if i % 2 == 0 else nc.scalar
            eng.dma_start(out_sl[b0 : b0 + CG], x_sl[b0 : b0 + CG])
    nc.compile()
    return nc


def kernel(x, scenario_gradient, mixup_strength, scenario, partner_idx, is_dominant):
    global LAST_RESULT
    from concourse.bass_utils import run_bass_kernel_spmd

    x = np.ascontiguousarray(np.asarray(x, dtype=np.float32))
    g = np.ascontiguousarray(np.asarray(scenario_gradient, dtype=np.float32))
    m = np.asarray(mixup_strength, dtype=np.float32).reshape(B, 1)
    p = np.asarray(partner_idx, dtype=np.int64).ravel()
    dm = np.asarray(is_dominant, dtype=bool).ravel()

    any_dom = bool(dm.any())
    key = "main" if any_dom else "copy"
    nc = _CACHE.get(key)
    if nc is None:
        nc = _build() if any_dom else _build_copy()
        _CACHE[key] = nc

    dom_f = dm.astype(np.float32).reshape(B, 1)
    p_eff = np.where(dm, p, np.arange(B, dtype=np.int64))
    # pmi = (P - I)^T with P[b, p_b] = 1: pmi[k, b] = [k == p_b] - [k == b]
    pmi = np.zeros((B, B), dtype=np.float32)
    pmi[p_eff, np.arange(B)] += 1.0
    pmi[np.arange(B), np.arange(B)] -= 1.0

    in_maps = []
    for c in range(N_CORES):
        sl = slice(c * T_LOC, (c + 1) * T_LOC)
        in_maps.append(
            {
                "x_sl": np.ascontiguousarray(x[:, sl, :]),
                "g_sl": np.ascontiguousarray(g[:, sl, :]),
                "m_in": m,
                "dom_in": dom_f,
                "pmi_in": pmi,
            }
        )

    res = run_bass_kernel_spmd(nc, in_maps, core_ids=list(range(N_CORES)))
    LAST_RESULT = res

    out = np.empty((B, T, D), dtype=np.float32)
    for c in range(N_CORES):
        out[:, c * T_LOC : (c + 1) * T_LOC, :] = res.results[c]["out_sl"]
    return out



# revision 7
# speedup vs baseline: 1.2791x; 1.2791x over previous
"""CAFE-interpolation kernel for 8 Trainium2 NeuronCores.

Primary strategy (B-shard, collective-free): partition the 128 samples
into 8 blocks of 16 such that every dominant sample's mixup partner
lands in the same block (host-side bin packing of the partner-graph
components). Each core then owns 16 samples over the FULL time axis, so
the quantile/mask/mixup is entirely core-local: no AllReduce, no
cross-core barrier, no entry-skew exposure.

Virtual-row layout: a core's [16 rows x 1024 t] slab is viewed as
[128 vrows, 128 ti, 512] with vrow v = r_local*8 + t_outer. All 128
SBUF partitions stay busy. The per-row sum over t then needs a final
8-way reduction across each vrow group, done with one PE matmul
(W[p,j] = 1/1024 iff p//8 == j//8), which also leaves im replicated
8x along partitions so the mask is directly in vrow layout.

Quantile (exact 52nd/53rd largest of 512) via per-row binary search on
the value axis: count = fused scalar_tensor_tensor(is_gt)+accum per
iteration, then exact v52/v53 extraction with masked reduces. Matches
jnp.quantile's interpolation; ties resolve to the identical mask.

Mixup: out = x + M^T @ (x * mask) with M[k,v] = c_v*([k==pv(v)]-[k==v])
(c folded into M, built host-side from metadata, passed as input).

Stage-3 reuses the tail of stage-1's x tiles still resident in SBUF
(xres pool bufs=18 -> 72 of 128 ti need no re-read).

Fallbacks (correct for any input): T-shard + CC AllReduce program when
the partner graph does not pack; pure-copy program when no sample is
dominant.
"""

import os
import numpy as np

B, T, D = 128, 1024, 512
N_CORES = 8
RPC = B // N_CORES  # 16 rows per core
TO = 8  # t_outer groups per row
TI = T // TO  # 128 t_inner steps
TG1 = 4  # stage-1 chunk: ti per chunk
NCH1 = TI // TG1  # 32 chunks
XRES_BUFS = 18  # stage-1 x tiles kept resident for stage 3 (72 ti)
TG3 = 2  # stage-3 t-pair
NIT = 24  # binary-search iterations
BIG = 1.0e30
KTH = 52.5  # count > KTH  <=>  count >= 53

# T-shard fallback constants (legacy program)
T_LOC = T // N_CORES
KTOP = 53
FTG1 = 8

_CACHE: dict = {}
LAST_RESULT = None


def _build_bshard():
    import concourse.mybir as mybir
    import concourse.tile as tile
    from concourse import bacc

    f32 = mybir.dt.float32
    Alu = mybir.AluOpType
    AX = mybir.AxisListType

    nc = bacc.Bacc(
        "TRN2", target_bir_lowering=False, debug=False, num_devices=N_CORES
    )
    x_in = nc.dram_tensor("x_vr", [B, TI, D], f32, kind="ExternalInput")
    g_in = nc.dram_tensor("g_vr", [B, TI, D], f32, kind="ExternalInput")
    m_in = nc.dram_tensor("m_mat", [B, B], f32, kind="ExternalInput")
    w_in = nc.dram_tensor("w_mat", [B, B], f32, kind="ExternalInput")
    out_vr = nc.dram_tensor("out_vr", [B, TI, D], f32, kind="ExternalOutput")

    with tile.TileContext(nc) as tc:
        with tc.tile_pool(name="persist", bufs=1) as pp:
            m_t = pp.tile([B, B], f32)
            nc.sync.dma_start(m_t[:], m_in[:])
            w_t = pp.tile([B, B], f32)
            nc.sync.dma_start(w_t[:], w_in[:])
            ones = pp.tile([B, D], f32)
            nc.vector.memset(ones[:], 1.0)
            imacc = pp.tile([B, D], f32)
            im_sb = pp.tile([B, D], f32)
            scr = pp.tile([B, D], f32)
            mask = pp.tile([B, D], f32)
            lo = pp.tile([B, 1], f32)
            wid = pp.tile([B, 1], f32)
            mid = pp.tile([B, 1], f32)
            cnt = pp.tile([B, 1], f32)
            ok = pp.tile([B, 1], f32)
            hi_f = pp.tile([B, 1], f32)
            v52 = pp.tile([B, 1], f32)
            v53 = pp.tile([B, 1], f32)
            thr = pp.tile([B, 1], f32)

            xres = []
            # ---- stage 1: imacc[v] = sum_ti x[v,ti,:]*g[v,ti,:] ----
            # xres stays open through stage 3 (tail tiles are reused there)
            xp_ctx = tc.tile_pool(name="xres", bufs=XRES_BUFS)
            xp = xp_ctx.__enter__()
            with (
                tc.tile_pool(name="g1", bufs=2) as gp,
                tc.tile_pool(name="prod", bufs=2) as prp,
                tc.tile_pool(name="tr1", bufs=2) as trp,
            ):
                for ci in range(NCH1):
                    t0 = ci * TG1
                    xt = xp.tile([B, TG1, D], f32, tag="x1")
                    gt = gp.tile([B, TG1, D], f32, tag="g1")
                    nc.sync.dma_start(xt[:], x_in[:, t0 : t0 + TG1, :])
                    nc.sync.dma_start(gt[:], g_in[:, t0 : t0 + TG1, :])
                    xres.append(xt)
                    prod = prp.tile([B, TG1, D], f32, tag="p1")
                    nc.vector.tensor_tensor(prod[:], xt[:], gt[:], op=Alu.mult)
                    # GpSimd cannot touch PSUM: DVE folds PSUM->SBUF, GpSimd
                    # finishes the SBUF-side tree + accumulation
                    f2 = trp.tile([B, 2, D], f32, tag="f2")
                    nc.vector.tensor_tensor(
                        f2[:], prod[:, 0:2, :], prod[:, 2:4, :], op=Alu.add
                    )
                    f1 = trp.tile([B, D], f32, tag="f1")
                    nc.gpsimd.tensor_tensor(
                        f1[:], f2[:, 0, :], f2[:, 1, :], op=Alu.add
                    )
                    if ci == 0:
                        nc.gpsimd.tensor_scalar(
                            imacc[:], f1[:], scalar1=0.0, scalar2=None, op0=Alu.add
                        )
                    else:
                        nc.gpsimd.tensor_tensor(
                            imacc[:], imacc[:], f1[:], op=Alu.add
                        )

            # ---- vrow-group reduction on PE: im_rep = W^T @ imacc ----
            with tc.tile_pool(name="psel", bufs=1, space="PSUM") as pselp:
                im_ps = pselp.tile([B, D], f32)
                nc.tensor.matmul(im_ps[:], w_t[:], imacc[:], start=True, stop=True)
                nc.vector.tensor_scalar(
                    im_sb[:], im_ps[:], scalar1=0.0, scalar2=None, op0=Alu.add
                )

            # ---- stage 2: binary-search the 52nd/53rd largest per row ----
            nc.vector.reduce_max(hi_f[:, 0:1], im_sb[:], axis=AX.X)
            nc.vector.tensor_reduce(
                lo[:, 0:1], im_sb[:], axis=AX.X, op=Alu.min
            )
            nc.vector.tensor_tensor(wid[:], hi_f[:], lo[:], op=Alu.subtract)
            for _ in range(NIT):
                # mid = lo + wid/2
                nc.vector.scalar_tensor_tensor(
                    mid[:], wid[:], 0.5, lo[:], op0=Alu.mult, op1=Alu.add
                )
                # cnt = sum(im > mid)
                nc.vector.scalar_tensor_tensor(
                    scr[:],
                    im_sb[:],
                    mid[:, 0:1],
                    ones[:],
                    op0=Alu.is_gt,
                    op1=Alu.mult,
                    accum_out=cnt[:, 0:1],
                )
                # ok = cnt >= 53 ; wid /= 2 ; lo += ok*wid
                nc.vector.tensor_scalar(
                    ok[:], cnt[:], scalar1=KTH, scalar2=None, op0=Alu.is_gt
                )
                nc.vector.tensor_scalar(
                    wid[:], wid[:], scalar1=0.5, scalar2=None, op0=Alu.mult
                )
                nc.vector.scalar_tensor_tensor(
                    lo[:], ok[:], wid[:, 0:1], lo[:], op0=Alu.mult, op1=Alu.add
                )
            # hi_f = lo + wid ;  invariant: count(>hi_f) <= 52, count(>lo) >= 53
            nc.vector.tensor_tensor(hi_f[:], lo[:], wid[:], op=Alu.add)
            # v52 = min{im > hi_f}: min(im + BIG*[im<=hi_f])
            nc.vector.tensor_scalar(
                scr[:],
                im_sb[:],
                scalar1=hi_f[:, 0:1],
                scalar2=BIG,
                op0=Alu.is_le,
                op1=Alu.mult,
            )
            nc.vector.tensor_tensor(scr[:], scr[:], im_sb[:], op=Alu.add)
            nc.vector.tensor_reduce(v52[:, 0:1], scr[:], axis=AX.X, op=Alu.min)
            # v53 = max{im <= hi_f}: max(im - BIG*[im>hi_f])
            nc.vector.tensor_scalar(
                scr[:],
                im_sb[:],
                scalar1=hi_f[:, 0:1],
                scalar2=BIG,
                op0=Alu.is_gt,
                op1=Alu.mult,
            )
            nc.vector.tensor_tensor(scr[:], im_sb[:], scr[:], op=Alu.subtract)
            nc.vector.reduce_max(v53[:, 0:1], scr[:], axis=AX.X)
            # thr = v53 + 0.9*(v52 - v53)  (jnp.quantile interpolation)
            nc.vector.tensor_tensor(thr[:], v52[:], v53[:], op=Alu.subtract)
            nc.vector.scalar_tensor_tensor(
                thr[:], thr[:], 0.9, v53[:], op0=Alu.mult, op1=Alu.add
            )
            nc.vector.tensor_scalar(
                mask[:], im_sb[:], scalar1=thr[:, 0:1], scalar2=None, op0=Alu.is_gt
            )

            # ---- stage 3: out = x + M^T @ (x*mask) ----
            n_res_ti = XRES_BUFS * TG1
            head_ti = TI - n_res_ti  # streamed ti count
            with (
                tc.tile_pool(name="x3", bufs=2) as x3p,
                tc.tile_pool(name="xm3", bufs=3) as xmp,
                tc.tile_pool(name="o3", bufs=3) as o3p,
                tc.tile_pool(name="q3", bufs=3, space="PSUM") as q3p,
            ):
                # (source-tile, local ti offset, global ti) per TG3-pair,
                # resident tail first, then streamed head
                pairs = []
                for ci in range(NCH1 - XRES_BUFS, NCH1):
                    for off in range(0, TG1, TG3):
                        pairs.append((xres[ci], off, ci * TG1 + off))
                head_tiles = {}
                for hc in range(0, head_ti, TG1):
                    head_tiles[hc] = None
                for hc in range(0, head_ti, TG1):
                    for off in range(0, TG1, TG3):
                        pairs.append((("head", hc), off, hc + off))

                for pi, (src, off, gti) in enumerate(pairs):
                    if isinstance(src, tuple):
                        hc = src[1]
                        if head_tiles[hc] is None:
                            ht = x3p.tile([B, TG1, D], f32, tag="x3")
                            nc.sync.dma_start(
                                ht[:], x_in[:, hc : hc + TG1, :]
                            )
                            head_tiles[hc] = ht
                        xt = head_tiles[hc]
                    else:
                        xt = src
                    xv = xt[:, off : off + TG3, :]
                    xm = xmp.tile([B, TG3, D], f32, tag="xm")
                    q = q3p.tile([B, TG3, D], f32, tag="q")
                    ot = o3p.tile([B, TG3, D], f32, tag="ot")
                    # q lives in PSUM -> the add must run on DVE; GpSimd takes
                    # most of the mask-muls to balance (DVE ~0.72us/512-op,
                    # GpSimd ~1.36us)
                    meng = nc.vector if pi % 10 < 3 else nc.gpsimd
                    for j in range(TG3):
                        meng.tensor_tensor(
                            xm[:, j, :], xv[:, j, :], mask[:], op=Alu.mult
                        )
                    for j in range(TG3):
                        nc.tensor.matmul(
                            q[:, j, :], m_t[:], xm[:, j, :], start=True, stop=True
                        )
                    nc.vector.tensor_tensor(ot[:], xv[:], q[:], op=Alu.add)
                    nc.scalar.dma_start(out_vr[:, gti : gti + TG3, :], ot[:])
            xp_ctx.__exit__(None, None, None)
    nc.compile()
    return nc


def _build_tshard():
    """Fallback: T-shard + CC AllReduce (handles any partner metadata)."""
    import concourse.mybir as mybir
    import concourse.tile as tile
    from concourse import bacc

    f32 = mybir.dt.float32
    Alu = mybir.AluOpType
    AX = mybir.AxisListType

    nc = bacc.Bacc(
        "TRN2", target_bir_lowering=False, debug=False, num_devices=N_CORES
    )
    x_sl = nc.dram_tensor("x_sl", [B, T_LOC, D], f32, kind="ExternalInput")
    g_sl = nc.dram_tensor("g_sl", [B, T_LOC, D], f32, kind="ExternalInput")
    m_in = nc.dram_tensor("m_in", [B, 1], f32, kind="ExternalInput")
    dom_in = nc.dram_tensor("dom_in", [B, 1], f32, kind="ExternalInput")
    pmi_in = nc.dram_tensor("pmi_in", [B, B], f32, kind="ExternalInput")
    out_sl = nc.dram_tensor("out_sl", [B, T_LOC, D], f32, kind="ExternalOutput")

    with tile.TileContext(nc) as tc:
        with tc.tile_pool(name="persist", bufs=1) as pp:
            m_t = pp.tile([B, 1], f32)
            nc.sync.dma_start(m_t[:], m_in[:])
            dom_t = pp.tile([B, 1], f32)
            nc.sync.dma_start(dom_t[:], dom_in[:])
            pmi_t = pp.tile([B, B], f32)
            nc.sync.dma_start(pmi_t[:], pmi_in[:])
            im_all = pp.tile([B, D], f32)
            cur_a = pp.tile([B, D], f32)
            cur_b = pp.tile([B, D], f32)
            mv = pp.tile([B, 64], f32)
            mask = pp.tile([B, D], f32)
            cvec = pp.tile([B, 1], f32)
            imacc = pp.tile([B, D], f32)

            with (
                tc.tile_pool(name="ld1", bufs=2) as ld1,
                tc.tile_pool(name="pr1", bufs=2) as pr1,
                tc.tile_pool(name="ccp", bufs=1, space="DRAM") as ccp,
            ):
                n_g1 = T_LOC // FTG1
                for i in range(n_g1):
                    t0 = i * FTG1
                    xt = ld1.tile([B, FTG1, D], f32, tag="x1")
                    gt = ld1.tile([B, FTG1, D], f32, tag="g1")
                    nc.sync.dma_start(xt[:], x_sl[:, t0 : t0 + FTG1, :])
                    nc.sync.dma_start(gt[:], g_sl[:, t0 : t0 + FTG1, :])
                    prod = pr1.tile([B, FTG1, D], f32, tag="prod")
                    nc.vector.tensor_tensor(prod[:], xt[:], gt[:], op=Alu.mult)
                    f4 = pr1.tile([B, FTG1 // 2, D], f32, tag="f4")
                    nc.vector.tensor_tensor(
                        f4[:], prod[:, 0 : FTG1 // 2, :], prod[:, FTG1 // 2 :, :],
                        op=Alu.add,
                    )
                    f2 = pr1.tile([B, FTG1 // 4, D], f32, tag="f2")
                    nc.vector.tensor_tensor(
                        f2[:], f4[:, 0 : FTG1 // 4, :], f4[:, FTG1 // 4 :, :],
                        op=Alu.add,
                    )
                    if i == 0:
                        nc.vector.tensor_tensor(
                            imacc[:], f2[:, 0, :], f2[:, 1, :], op=Alu.add
                        )
                    else:
                        part = pr1.tile([B, D], f32, tag="part")
                        nc.vector.tensor_tensor(
                            part[:], f2[:, 0, :], f2[:, 1, :], op=Alu.add
                        )
                        nc.vector.tensor_tensor(
                            imacc[:], imacc[:], part[:], op=Alu.add
                        )
                nc.vector.tensor_scalar(
                    imacc[:], imacc[:], scalar1=1.0 / T, scalar2=None, op0=Alu.mult
                )

                cc_in_t = ccp.tile([B, D], f32, name="cc_in_t")
                cc_out_t = ccp.tile([B, D], f32, name="cc_out_t")
                nc.gpsimd.dma_start(cc_in_t[:], imacc[:])
                nc.gpsimd.collective_compute(
                    "AllReduce",
                    Alu.add,
                    replica_groups=[list(range(N_CORES))],
                    ins=[cc_in_t.opt()],
                    outs=[cc_out_t.opt()],
                )
                nc.gpsimd.dma_start(im_all[:], cc_out_t[:])

            with (
                tc.tile_pool(name="sel", bufs=2) as selp,  # noqa: F841
                tc.tile_pool(name="psumw", bufs=1, space="PSUM") as psumw,
            ):
                cur, nxt = im_all, cur_b
                nc.vector.reduce_max(mv[:, 0:1], cur[:], axis=AX.X)
                for k in range(1, KTOP):
                    nc.vector.scalar_tensor_tensor(
                        nxt[:],
                        cur[:],
                        mv[:, k - 1 : k],
                        cur[:],
                        op0=Alu.is_lt,
                        op1=Alu.mult,
                    )
                    nc.vector.reduce_max(mv[:, k : k + 1], nxt[:], axis=AX.X)
                    cur = nxt
                    nxt = cur_a if cur is cur_b else cur_b

                qw = psumw.tile([B, D], f32)
                for _ in range(20):
                    nc.tensor.matmul(
                        qw[:], pmi_t[:], im_all[:], start=True, stop=True
                    )

                dl = pp.tile([B, 1], f32)
                nc.vector.tensor_tensor(
                    dl[:], mv[:, 51:52], mv[:, 52:53], op=Alu.subtract
                )
                dl9 = pp.tile([B, 1], f32)
                nc.vector.tensor_scalar(
                    dl9[:], dl[:], scalar1=0.9, scalar2=None, op0=Alu.mult
                )
                thr_t = pp.tile([B, 1], f32)
                nc.vector.tensor_tensor(thr_t[:], mv[:, 52:53], dl9[:], op=Alu.add)

                nc.vector.tensor_scalar(
                    mask[:],
                    im_all[:],
                    scalar1=thr_t[:, 0:1],
                    scalar2=None,
                    op0=Alu.is_gt,
                )

                om_t = pp.tile([B, 1], f32)
                nc.vector.tensor_scalar(
                    om_t[:],
                    m_t[:],
                    scalar1=-1.0,
                    scalar2=1.0,
                    op0=Alu.mult,
                    op1=Alu.add,
                )
                nc.vector.tensor_tensor(cvec[:], om_t[:], dom_t[:], op=Alu.mult)

            with (
                tc.tile_pool(name="x3", bufs=36) as x3p,
                tc.tile_pool(name="t3", bufs=4) as t3p,
                tc.tile_pool(name="psumq", bufs=3, space="PSUM") as psumq,
            ):
                for gi, t0 in enumerate(range(0, T_LOC, TG3)):
                    xt3 = x3p.tile([B, TG3, D], f32, tag="x3t")
                    nc.sync.dma_start(xt3[:], x_sl[:, t0 : t0 + TG3, :])
                    q = psumq.tile([B, TG3, D], f32, tag="q")
                    ot = t3p.tile([B, TG3, D], f32, tag="ot")
                    xm = t3p.tile([B, TG3, D], f32, tag="xm")
                    eng = nc.vector if gi % 2 == 0 else nc.gpsimd
                    for j in range(TG3):
                        eng.tensor_tensor(
                            xm[:, j, :], xt3[:, j, :], mask[:], op=Alu.mult
                        )
                    for j in range(TG3):
                        nc.tensor.matmul(
                            q[:, j, :], pmi_t[:], xm[:, j, :], start=True, stop=True
                        )
                    nc.vector.scalar_tensor_tensor(
                        ot[:],
                        q[:],
                        cvec[:, 0:1],
                        xt3[:],
                        op0=Alu.mult,
                        op1=Alu.add,
                    )
                    nc.scalar.dma_start(out_sl[:, t0 : t0 + TG3, :], ot[:])
    nc.compile()
    return nc


def _build_copy():
    """All-non-dominant fast path: output == x."""
    import concourse.mybir as mybir
    import concourse.tile as tile
    from concourse import bacc

    f32 = mybir.dt.float32
    nc = bacc.Bacc(
        "TRN2", target_bir_lowering=False, debug=False, num_devices=N_CORES
    )
    x_sl = nc.dram_tensor("x_sl", [B, T_LOC, D], f32, kind="ExternalInput")
    nc.dram_tensor("g_sl", [B, T_LOC, D], f32, kind="ExternalInput")
    nc.dram_tensor("m_in", [B, 1], f32, kind="ExternalInput")
    nc.dram_tensor("dom_in", [B, 1], f32, kind="ExternalInput")
    nc.dram_tensor("pmi_in", [B, B], f32, kind="ExternalInput")
    out_sl = nc.dram_tensor("out_sl", [B, T_LOC, D], f32, kind="ExternalOutput")
    with tile.TileContext(nc):
        CG = 8
        for i, b0 in enumerate(range(0, B, CG)):
            eng = nc.sync if i % 2 == 0 else nc.scalar
            eng.dma_start(out_sl[b0 : b0 + CG], x_sl[b0 : b0 + CG])
    nc.compile()
    return nc


def _pack_blocks(p_eff, dm):
    """Partition samples into 8 blocks of 16 with every dominant sample's
    partner co-located. Returns list of 8 lists, or None if impossible."""
    parent = list(range(B))

    def find(a):
        while parent[a] != a:
            parent[a] = parent[parent[a]]
            a = parent[a]
        return a

    for b in range(B):
        if dm[b]:
            ra, rb = find(b), find(int(p_eff[b]))
            if ra != rb:
                parent[ra] = rb
    comps: dict = {}
    for i in range(B):
        comps.setdefault(find(i), []).append(i)
    comp_list = sorted(comps.values(), key=len, reverse=True)
    if len(comp_list[0]) > RPC:
        return None
    bins = [[] for _ in range(N_CORES)]
    for comp in comp_list:
        comp_bins = sorted(bins, key=len)
        placed = False
        for bn in comp_bins:
            if len(bn) + len(comp) <= RPC:
                bn.extend(comp)
                placed = True
                break
        if not placed:
            return None
    if any(len(bn) != RPC for bn in bins):
        return None
    return bins


def kernel(x, scenario_gradient, mixup_strength, scenario, partner_idx, is_dominant):
    global LAST_RESULT
    from concourse.bass_utils import run_bass_kernel_spmd

    x = np.ascontiguousarray(np.asarray(x, dtype=np.float32))
    g = np.ascontiguousarray(np.asarray(scenario_gradient, dtype=np.float32))
    m = np.asarray(mixup_strength, dtype=np.float32).ravel()
    p = np.asarray(partner_idx, dtype=np.int64).ravel()
    dm = np.asarray(is_dominant, dtype=bool).ravel()

    any_dom = bool(dm.any())
    p_eff = np.where(dm, p, np.arange(B, dtype=np.int64))
    cvec = np.where(dm, 1.0 - m, 0.0).astype(np.float32)
    blocks = _pack_blocks(p_eff, dm) if any_dom else None

    if not any_dom:
        return _run_copy(x, g, m, dm, p_eff)
    if blocks is None:
        return _run_tshard(x, g, m, dm, p_eff)

    nc = _CACHE.get("bshard")
    if nc is None:
        nc = _build_bshard()
        _CACHE["bshard"] = nc

    # W: vrow-group summation (+ 1/T mean fold), same for every core
    wmat = np.zeros((B, B), dtype=np.float32)
    for j in range(B):
        g0 = (j // TO) * TO
        wmat[g0 : g0 + TO, j] = 1.0 / T
    wmat = np.ascontiguousarray(wmat)

    in_maps = []
    for c in range(N_CORES):
        rows = blocks[c]
        loc = {r: i for i, r in enumerate(rows)}
        x_vr = np.ascontiguousarray(
            x[rows].reshape(RPC, TO, TI, D).reshape(B, TI, D)
        )
        g_vr = np.ascontiguousarray(
            g[rows].reshape(RPC, TO, TI, D).reshape(B, TI, D)
        )
        mmat = np.zeros((B, B), dtype=np.float32)
        for i, r in enumerate(rows):
            c_r = float(cvec[r])
            if c_r == 0.0:
                continue
            pl = loc[int(p_eff[r])]
            for to in range(TO):
                v = i * TO + to
                pv = pl * TO + to
                mmat[pv, v] += c_r
                mmat[v, v] -= c_r
        in_maps.append(
            {
                "x_vr": x_vr,
                "g_vr": g_vr,
                "m_mat": np.ascontiguousarray(mmat),
                "w_mat": wmat,
            }
        )

    res = run_bass_kernel_spmd(nc, in_maps, core_ids=list(range(N_CORES)))
    LAST_RESULT = res

    out = np.empty((B, T, D), dtype=np.float32)
    for c in range(N_CORES):
        o = res.results[c]["out_vr"].reshape(RPC, TO, TI, D).reshape(RPC, T, D)
        out[blocks[c]] = o
    return out


def _run_tshard(x, g, m, dm, p_eff):
    global LAST_RESULT
    from concourse.bass_utils import run_bass_kernel_spmd

    nc = _CACHE.get("tshard")
    if nc is None:
        nc = _build_tshard()
        _CACHE["tshard"] = nc
    dom_f = dm.astype(np.float32).reshape(B, 1)
    pmi = np.zeros((B, B), dtype=np.float32)
    pmi[p_eff, np.arange(B)] += 1.0
    pmi[np.arange(B), np.arange(B)] -= 1.0
    in_maps = []
    for c in range(N_CORES):
        sl = slice(c * T_LOC, (c + 1) * T_LOC)
        in_maps.append(
            {
                "x_sl": np.ascontiguousarray(x[:, sl, :]),
                "g_sl": np.ascontiguousarray(g[:, sl, :]),
                "m_in": m.reshape(B, 1),
                "dom_in": dom_f,
                "pmi_in": pmi,
            }
        )
    res = run_bass_kernel_spmd(nc, in_maps, core_ids=list(range(N_CORES)))
    LAST_RESULT = res
    out = np.empty((B, T, D), dtype=np.float32)
    for c in range(N_CORES):
        out[:, c * T_LOC : (c + 1) * T_LOC, :] = res.results[c]["out_sl"]
    return out


def _run_copy(x, g, m, dm, p_eff):
    global LAST_RESULT
    from concourse.bass_utils import run_bass_kernel_spmd

    nc = _CACHE.get("copy")
    if nc is None:
        nc = _build_copy()
        _CACHE["copy"] = nc
    dom_f = dm.astype(np.float32).reshape(B, 1)
    pmi = np.zeros((B, B), dtype=np.float32)
    in_maps = []
    for c in range(N_CORES):
        sl = slice(c * T_LOC, (c + 1) * T_LOC)
        in_maps.append(
            {
                "x_sl": np.ascontiguousarray(x[:, sl, :]),
                "g_sl": np.ascontiguousarray(g[:, sl, :]),
                "m_in": m.reshape(B, 1),
                "dom_in": dom_f,
                "pmi_in": pmi,
            }
        )
    res = run_bass_kernel_spmd(nc, in_maps, core_ids=list(range(N_CORES)))
    LAST_RESULT = res
    out = np.empty((B, T, D), dtype=np.float32)
    for c in range(N_CORES):
        out[:, c * T_LOC : (c + 1) * T_LOC, :] = res.results[c]["out_sl"]
    return out


# revision 11
# speedup vs baseline: 1.3465x; 1.0528x over previous
"""CAFE-interpolation kernel for 8 Trainium2 NeuronCores.

Primary strategy (B-shard, collective-free): partition the 128 samples
into 8 blocks of 16 such that every dominant sample's mixup partner
lands in the same block (host-side bin packing of the partner-graph
components). Each core then owns 16 samples over the FULL time axis, so
the quantile/mask/mixup is entirely core-local: no AllReduce, no
cross-core barrier, no entry-skew exposure.

Virtual-row layout: a core's [16 rows x 1024 t] slab is viewed as
[128 vrows, 128 ti, 512] with vrow v = r_local*8 + t_outer. All 128
SBUF partitions stay busy. The per-row sum over t then needs a final
8-way reduction across each vrow group, done with one PE matmul
(W[p,j] = 1/1024 iff p//8 == j//8), which also leaves im replicated
8x along partitions so the mask is directly in vrow layout.

Quantile (exact 52nd/53rd largest of 512) via per-row binary search on
the value axis: count = fused scalar_tensor_tensor(is_gt)+accum per
iteration, then exact v52/v53 extraction with masked reduces. Matches
jnp.quantile's interpolation; ties resolve to the identical mask.

Mixup: out = x + M^T @ (x * mask) with M[k,v] = c_v*([k==pv(v)]-[k==v])
(c folded into M, built host-side from metadata, passed as input).

Stage-3 reuses the tail of stage-1's x tiles still resident in SBUF
(xres pool bufs=18 -> 72 of 128 ti need no re-read).

Fallbacks (correct for any input): T-shard + CC AllReduce program when
the partner graph does not pack; pure-copy program when no sample is
dominant.
"""

import os
import numpy as np

B, T, D = 128, 1024, 512
N_CORES = 8
RPC = B // N_CORES  # 16 rows per core
TO = 8  # t_outer groups per row
TI = T // TO  # 128 t_inner steps
TG1 = 8  # stage-1 chunk: ti per chunk (16KB/partition DMA lines)
NCH1 = TI // TG1  # 16 chunks
XRES_BUFS = 8  # stage-1 x tiles kept resident for stage 3 (64 ti)
TG3 = 4  # stage-3 t-group
NIT = 20  # binary-search iterations
BIG = 1.0e30
KTH = 52.5  # count > KTH  <=>  count >= 53

# T-shard fallback constants (legacy program)
T_LOC = T // N_CORES
KTOP = 53
FTG1 = 8

_CACHE: dict = {}
LAST_RESULT = None


def _build_bshard():
    import concourse.mybir as mybir
    import concourse.tile as tile
    from concourse import bacc

    f32 = mybir.dt.float32
    Alu = mybir.AluOpType
    AX = mybir.AxisListType

    nc = bacc.Bacc(
        "TRN2", target_bir_lowering=False, debug=False, num_devices=N_CORES
    )
    x_in = nc.dram_tensor("x_vr", [B, TI, D], f32, kind="ExternalInput")
    g_in = nc.dram_tensor("g_vr", [B, TI, D], f32, kind="ExternalInput")
    m_in = nc.dram_tensor("m_mat", [B, B], f32, kind="ExternalInput")
    w_in = nc.dram_tensor("w_mat", [B, B], f32, kind="ExternalInput")
    out_vr = nc.dram_tensor("out_vr", [B, TI, D], f32, kind="ExternalOutput")

    with tile.TileContext(nc) as tc:
        with tc.tile_pool(name="persist", bufs=1) as pp:
            m_t = pp.tile([B, B], f32)
            nc.sync.dma_start(m_t[:], m_in[:])
            w_t = pp.tile([B, B], f32)
            nc.sync.dma_start(w_t[:], w_in[:])
            ones = pp.tile([B, D], f32)
            nc.vector.memset(ones[:], 1.0)
            im_sb = pp.tile([B, D], f32)
            scr = pp.tile([B, D], f32)
            mask = pp.tile([B, D], f32)
            lo = pp.tile([B, 1], f32)
            wid = pp.tile([B, 1], f32)
            mid = pp.tile([B, 1], f32)
            cnt = pp.tile([B, 1], f32)
            ok = pp.tile([B, 1], f32)
            hi_f = pp.tile([B, 1], f32)
            v52 = pp.tile([B, 1], f32)
            v53 = pp.tile([B, 1], f32)
            thr = pp.tile([B, 1], f32)

            xres = []
            # ---- stage 1: im = (1/T) sum_t x*g, reduced entirely on PE ----
            # DVE/GpSimd only compute prod = x*g; the PE accumulates
            # W^T @ prod[:,j,:] over all 128 (chunk, j) into one PSUM bank
            # (W also folds the vrow-group sum and the 1/T mean).
            # xres stays open through stage 3 (tail tiles are reused there).
            xp_ctx = tc.tile_pool(name="xres", bufs=XRES_BUFS)
            xp = xp_ctx.__enter__()
            with (
                tc.tile_pool(name="g1", bufs=2) as gp,
                tc.tile_pool(name="prod", bufs=2) as prp,
                tc.tile_pool(name="psel", bufs=1, space="PSUM") as pselp,
            ):
                im_ps = pselp.tile([B, D], f32)
                for ci in range(NCH1):
                    t0 = ci * TG1
                    xt = xp.tile([B, TG1, D], f32, tag="x1")
                    gt = gp.tile([B, TG1, D], f32, tag="g1")
                    nc.sync.dma_start(xt[:], x_in[:, t0 : t0 + TG1, :])
                    nc.sync.dma_start(gt[:], g_in[:, t0 : t0 + TG1, :])
                    xres.append(xt)
                    prod = prp.tile([B, TG1, D], f32, tag="p1")
                    peng = nc.vector if ci % 2 == 0 else nc.gpsimd
                    peng.tensor_tensor(prod[:], xt[:], gt[:], op=Alu.mult)
                    for j in range(TG1):
                        nc.tensor.matmul(
                            im_ps[:],
                            w_t[:],
                            prod[:, j, :],
                            start=(ci == 0 and j == 0),
                            stop=(ci == NCH1 - 1 and j == TG1 - 1),
                        )
                nc.vector.tensor_scalar(
                    im_sb[:], im_ps[:], scalar1=0.0, scalar2=None, op0=Alu.add
                )

            # ---- stage 2: binary-search the 52nd/53rd largest per row ----
            nc.vector.reduce_max(hi_f[:, 0:1], im_sb[:], axis=AX.X)
            nc.vector.tensor_reduce(
                lo[:, 0:1], im_sb[:], axis=AX.X, op=Alu.min
            )
            nc.vector.tensor_tensor(wid[:], hi_f[:], lo[:], op=Alu.subtract)
            for _ in range(NIT):
                # mid = lo + wid/2
                nc.vector.scalar_tensor_tensor(
                    mid[:], wid[:], 0.5, lo[:], op0=Alu.mult, op1=Alu.add
                )
                # cnt = sum(im > mid)
                nc.vector.scalar_tensor_tensor(
                    scr[:],
                    im_sb[:],
                    mid[:, 0:1],
                    ones[:],
                    op0=Alu.is_gt,
                    op1=Alu.mult,
                    accum_out=cnt[:, 0:1],
                )
                # ok = cnt >= 53 ; wid /= 2 ; lo += ok*wid
                nc.vector.tensor_scalar(
                    ok[:], cnt[:], scalar1=KTH, scalar2=None, op0=Alu.is_gt
                )
                nc.vector.tensor_scalar(
                    wid[:], wid[:], scalar1=0.5, scalar2=None, op0=Alu.mult
                )
                nc.vector.scalar_tensor_tensor(
                    lo[:], ok[:], wid[:, 0:1], lo[:], op0=Alu.mult, op1=Alu.add
                )
            # hi_f = lo + wid ;  invariant: count(>hi_f) <= 52, count(>lo) >= 53
            nc.vector.tensor_tensor(hi_f[:], lo[:], wid[:], op=Alu.add)
            # v52 = min{im > hi_f}: min(im + BIG*[im<=hi_f])
            nc.vector.tensor_scalar(
                scr[:],
                im_sb[:],
                scalar1=hi_f[:, 0:1],
                scalar2=BIG,
                op0=Alu.is_le,
                op1=Alu.mult,
            )
            nc.vector.tensor_tensor(scr[:], scr[:], im_sb[:], op=Alu.add)
            nc.vector.tensor_reduce(v52[:, 0:1], scr[:], axis=AX.X, op=Alu.min)
            # v53 = max{im <= hi_f}: max(im - BIG*[im>hi_f])
            nc.vector.tensor_scalar(
                scr[:],
                im_sb[:],
                scalar1=hi_f[:, 0:1],
                scalar2=BIG,
                op0=Alu.is_gt,
                op1=Alu.mult,
            )
            nc.vector.tensor_tensor(scr[:], im_sb[:], scr[:], op=Alu.subtract)
            nc.vector.reduce_max(v53[:, 0:1], scr[:], axis=AX.X)
            # thr = v53 + 0.9*(v52 - v53)  (jnp.quantile interpolation)
            nc.vector.tensor_tensor(thr[:], v52[:], v53[:], op=Alu.subtract)
            nc.vector.scalar_tensor_tensor(
                thr[:], thr[:], 0.9, v53[:], op0=Alu.mult, op1=Alu.add
            )
            nc.vector.tensor_scalar(
                mask[:], im_sb[:], scalar1=thr[:, 0:1], scalar2=None, op0=Alu.is_gt
            )

            # ---- stage 3: out = x + M^T @ (x*mask) ----
            # GpSimd does all mask-muls (batched [B,TG3,D] ops amortize its
            # ~0.9us fixed overhead); DVE does all PSUM-reading adds.
            n_res_ti = XRES_BUFS * TG1
            head_ti = TI - n_res_ti  # streamed ti count
            with (
                tc.tile_pool(name="x3", bufs=2) as x3p,
                tc.tile_pool(name="xm3", bufs=2) as xmp,
                tc.tile_pool(name="o3", bufs=2) as o3p,
                tc.tile_pool(name="q3", bufs=2, space="PSUM") as q3p,
            ):
                steps = []
                for ci in range(NCH1 - XRES_BUFS, NCH1):
                    for off in range(0, TG1, TG3):
                        steps.append((xres[ci], off, ci * TG1 + off))
                head_tiles: dict = {}
                for hc in range(0, head_ti, TG1):
                    head_tiles[hc] = None
                    for off in range(0, TG1, TG3):
                        steps.append((("head", hc), off, hc + off))

                for src, off, gti in steps:
                    if isinstance(src, tuple):
                        hc = src[1]
                        if head_tiles[hc] is None:
                            ht = x3p.tile([B, TG1, D], f32, tag="x3")
                            nc.sync.dma_start(
                                ht[:], x_in[:, hc : hc + TG1, :]
                            )
                            head_tiles[hc] = ht
                        xt = head_tiles[hc]
                    else:
                        xt = src
                    xv = xt[:, off : off + TG3, :]
                    xm = xmp.tile([B, TG3, D], f32, tag="xm")
                    q = q3p.tile([B, TG3, D], f32, tag="q")
                    ot = o3p.tile([B, TG3, D], f32, tag="ot")
                    nc.gpsimd.tensor_tensor(
                        xm[:],
                        xv[:],
                        mask[:, None, :].broadcast_to([B, TG3, D]),
                        op=Alu.mult,
                    )
                    for j in range(TG3):
                        nc.tensor.matmul(
                            q[:, j, :], m_t[:], xm[:, j, :], start=True, stop=True
                        )
                    nc.vector.tensor_tensor(ot[:], xv[:], q[:], op=Alu.add)
                    nc.scalar.dma_start(out_vr[:, gti : gti + TG3, :], ot[:])
            xp_ctx.__exit__(None, None, None)
    nc.compile()
    return nc


def _build_tshard():
    """Fallback: T-shard + CC AllReduce (handles any partner metadata)."""
    import concourse.mybir as mybir
    import concourse.tile as tile
    from concourse import bacc

    f32 = mybir.dt.float32
    Alu = mybir.AluOpType
    AX = mybir.AxisListType

    nc = bacc.Bacc(
        "TRN2", target_bir_lowering=False, debug=False, num_devices=N_CORES
    )
    x_sl = nc.dram_tensor("x_sl", [B, T_LOC, D], f32, kind="ExternalInput")
    g_sl = nc.dram_tensor("g_sl", [B, T_LOC, D], f32, kind="ExternalInput")
    m_in = nc.dram_tensor("m_in", [B, 1], f32, kind="ExternalInput")
    dom_in = nc.dram_tensor("dom_in", [B, 1], f32, kind="ExternalInput")
    pmi_in = nc.dram_tensor("pmi_in", [B, B], f32, kind="ExternalInput")
    out_sl = nc.dram_tensor("out_sl", [B, T_LOC, D], f32, kind="ExternalOutput")

    with tile.TileContext(nc) as tc:
        with tc.tile_pool(name="persist", bufs=1) as pp:
            m_t = pp.tile([B, 1], f32)
            nc.sync.dma_start(m_t[:], m_in[:])
            dom_t = pp.tile([B, 1], f32)
            nc.sync.dma_start(dom_t[:], dom_in[:])
            pmi_t = pp.tile([B, B], f32)
            nc.sync.dma_start(pmi_t[:], pmi_in[:])
            im_all = pp.tile([B, D], f32)
            cur_a = pp.tile([B, D], f32)
            cur_b = pp.tile([B, D], f32)
            mv = pp.tile([B, 64], f32)
            mask = pp.tile([B, D], f32)
            cvec = pp.tile([B, 1], f32)
            imacc = pp.tile([B, D], f32)

            with (
                tc.tile_pool(name="ld1", bufs=2) as ld1,
                tc.tile_pool(name="pr1", bufs=2) as pr1,
                tc.tile_pool(name="ccp", bufs=1, space="DRAM") as ccp,
            ):
                n_g1 = T_LOC // FTG1
                for i in range(n_g1):
                    t0 = i * FTG1
                    xt = ld1.tile([B, FTG1, D], f32, tag="x1")
                    gt = ld1.tile([B, FTG1, D], f32, tag="g1")
                    nc.sync.dma_start(xt[:], x_sl[:, t0 : t0 + FTG1, :])
                    nc.sync.dma_start(gt[:], g_sl[:, t0 : t0 + FTG1, :])
                    prod = pr1.tile([B, FTG1, D], f32, tag="prod")
                    nc.vector.tensor_tensor(prod[:], xt[:], gt[:], op=Alu.mult)
                    f4 = pr1.tile([B, FTG1 // 2, D], f32, tag="f4")
                    nc.vector.tensor_tensor(
                        f4[:], prod[:, 0 : FTG1 // 2, :], prod[:, FTG1 // 2 :, :],
                        op=Alu.add,
                    )
                    f2 = pr1.tile([B, FTG1 // 4, D], f32, tag="f2")
                    nc.vector.tensor_tensor(
                        f2[:], f4[:, 0 : FTG1 // 4, :], f4[:, FTG1 // 4 :, :],
                        op=Alu.add,
                    )
                    if i == 0:
                        nc.vector.tensor_tensor(
                            imacc[:], f2[:, 0, :], f2[:, 1, :], op=Alu.add
                        )
                    else:
                        part = pr1.tile([B, D], f32, tag="part")
                        nc.vector.tensor_tensor(
                            part[:], f2[:, 0, :], f2[:, 1, :], op=Alu.add
                        )
                        nc.vector.tensor_tensor(
                            imacc[:], imacc[:], part[:], op=Alu.add
                        )
                nc.vector.tensor_scalar(
                    imacc[:], imacc[:], scalar1=1.0 / T, scalar2=None, op0=Alu.mult
                )

                cc_in_t = ccp.tile([B, D], f32, name="cc_in_t")
                cc_out_t = ccp.tile([B, D], f32, name="cc_out_t")
                nc.gpsimd.dma_start(cc_in_t[:], imacc[:])
                nc.gpsimd.collective_compute(
                    "AllReduce",
                    Alu.add,
                    replica_groups=[list(range(N_CORES))],
                    ins=[cc_in_t.opt()],
                    outs=[cc_out_t.opt()],
                )
                nc.gpsimd.dma_start(im_all[:], cc_out_t[:])

            with (
                tc.tile_pool(name="sel", bufs=2) as selp,  # noqa: F841
                tc.tile_pool(name="psumw", bufs=1, space="PSUM") as psumw,
            ):
                cur, nxt = im_all, cur_b
                nc.vector.reduce_max(mv[:, 0:1], cur[:], axis=AX.X)
                for k in range(1, KTOP):
                    nc.vector.scalar_tensor_tensor(
                        nxt[:],
                        cur[:],
                        mv[:, k - 1 : k],
                        cur[:],
                        op0=Alu.is_lt,
                        op1=Alu.mult,
                    )
                    nc.vector.reduce_max(mv[:, k : k + 1], nxt[:], axis=AX.X)
                    cur = nxt
                    nxt = cur_a if cur is cur_b else cur_b

                qw = psumw.tile([B, D], f32)
                for _ in range(20):
                    nc.tensor.matmul(
                        qw[:], pmi_t[:], im_all[:], start=True, stop=True
                    )

                dl = pp.tile([B, 1], f32)
                nc.vector.tensor_tensor(
                    dl[:], mv[:, 51:52], mv[:, 52:53], op=Alu.subtract
                )
                dl9 = pp.tile([B, 1], f32)
                nc.vector.tensor_scalar(
                    dl9[:], dl[:], scalar1=0.9, scalar2=None, op0=Alu.mult
                )
                thr_t = pp.tile([B, 1], f32)
                nc.vector.tensor_tensor(thr_t[:], mv[:, 52:53], dl9[:], op=Alu.add)

                nc.vector.tensor_scalar(
                    mask[:],
                    im_all[:],
                    scalar1=thr_t[:, 0:1],
                    scalar2=None,
                    op0=Alu.is_gt,
                )

                om_t = pp.tile([B, 1], f32)
                nc.vector.tensor_scalar(
                    om_t[:],
                    m_t[:],
                    scalar1=-1.0,
                    scalar2=1.0,
                    op0=Alu.mult,
                    op1=Alu.add,
                )
                nc.vector.tensor_tensor(cvec[:], om_t[:], dom_t[:], op=Alu.mult)

            with (
                tc.tile_pool(name="x3", bufs=36) as x3p,
                tc.tile_pool(name="t3", bufs=4) as t3p,
                tc.tile_pool(name="psumq", bufs=3, space="PSUM") as psumq,
            ):
                for gi, t0 in enumerate(range(0, T_LOC, TG3)):
                    xt3 = x3p.tile([B, TG3, D], f32, tag="x3t")
                    nc.sync.dma_start(xt3[:], x_sl[:, t0 : t0 + TG3, :])
                    q = psumq.tile([B, TG3, D], f32, tag="q")
                    ot = t3p.tile([B, TG3, D], f32, tag="ot")
                    xm = t3p.tile([B, TG3, D], f32, tag="xm")
                    eng = nc.vector if gi % 2 == 0 else nc.gpsimd
                    for j in range(TG3):
                        eng.tensor_tensor(
                            xm[:, j, :], xt3[:, j, :], mask[:], op=Alu.mult
                        )
                    for j in range(TG3):
                        nc.tensor.matmul(
                            q[:, j, :], pmi_t[:], xm[:, j, :], start=True, stop=True
                        )
                    nc.vector.scalar_tensor_tensor(
                        ot[:],
                        q[:],
                        cvec[:, 0:1],
                        xt3[:],
                        op0=Alu.mult,
                        op1=Alu.add,
                    )
                    nc.scalar.dma_start(out_sl[:, t0 : t0 + TG3, :], ot[:])
    nc.compile()
    return nc


def _build_copy():
    """All-non-dominant fast path: output == x."""
    import concourse.mybir as mybir
    import concourse.tile as tile
    from concourse import bacc

    f32 = mybir.dt.float32
    nc = bacc.Bacc(
        "TRN2", target_bir_lowering=False, debug=False, num_devices=N_CORES
    )
    x_sl = nc.dram_tensor("x_sl", [B, T_LOC, D], f32, kind="ExternalInput")
    nc.dram_tensor("g_sl", [B, T_LOC, D], f32, kind="ExternalInput")
    nc.dram_tensor("m_in", [B, 1], f32, kind="ExternalInput")
    nc.dram_tensor("dom_in", [B, 1], f32, kind="ExternalInput")
    nc.dram_tensor("pmi_in", [B, B], f32, kind="ExternalInput")
    out_sl = nc.dram_tensor("out_sl", [B, T_LOC, D], f32, kind="ExternalOutput")
    with tile.TileContext(nc):
        CG = 8
        for i, b0 in enumerate(range(0, B, CG)):
            eng = nc.sync if i % 2 == 0 else nc.scalar
            eng.dma_start(out_sl[b0 : b0 + CG], x_sl[b0 : b0 + CG])
    nc.compile()
    return nc


def _pack_blocks(p_eff, dm):
    """Partition samples into 8 blocks of 16 with every dominant sample's
    partner co-located. Returns list of 8 lists, or None if impossible."""
    parent = list(range(B))

    def find(a):
        while parent[a] != a:
            parent[a] = parent[parent[a]]
            a = parent[a]
        return a

    for b in range(B):
        if dm[b]:
            ra, rb = find(b), find(int(p_eff[b]))
            if ra != rb:
                parent[ra] = rb
    comps: dict = {}
    for i in range(B):
        comps.setdefault(find(i), []).append(i)
    comp_list = sorted(comps.values(), key=len, reverse=True)
    if len(comp_list[0]) > RPC:
        return None
    bins = [[] for _ in range(N_CORES)]
    for comp in comp_list:
        comp_bins = sorted(bins, key=len)
        placed = False
        for bn in comp_bins:
            if len(bn) + len(comp) <= RPC:
                bn.extend(comp)
                placed = True
                break
        if not placed:
            return None
    if any(len(bn) != RPC for bn in bins):
        return None
    return bins


def kernel(x, scenario_gradient, mixup_strength, scenario, partner_idx, is_dominant):
    global LAST_RESULT
    from concourse.bass_utils import run_bass_kernel_spmd

    x = np.ascontiguousarray(np.asarray(x, dtype=np.float32))
    g = np.ascontiguousarray(np.asarray(scenario_gradient, dtype=np.float32))
    m = np.asarray(mixup_strength, dtype=np.float32).ravel()
    p = np.asarray(partner_idx, dtype=np.int64).ravel()
    dm = np.asarray(is_dominant, dtype=bool).ravel()

    any_dom = bool(dm.any())
    p_eff = np.where(dm, p, np.arange(B, dtype=np.int64))
    cvec = np.where(dm, 1.0 - m, 0.0).astype(np.float32)
    blocks = _pack_blocks(p_eff, dm) if any_dom else None

    if not any_dom:
        return _run_copy(x, g, m, dm, p_eff)
    if blocks is None:
        return _run_tshard(x, g, m, dm, p_eff)

    nc = _CACHE.get("bshard")
    if nc is None:
        nc = _build_bshard()
        _CACHE["bshard"] = nc

    # W: vrow-group summation (+ 1/T mean fold), same for every core
    wmat = np.zeros((B, B), dtype=np.float32)
    for j in range(B):
        g0 = (j // TO) * TO
        wmat[g0 : g0 + TO, j] = 1.0 / T
    wmat = np.ascontiguousarray(wmat)

    in_maps = []
    for c in range(N_CORES):
        rows = blocks[c]
        loc = {r: i for i, r in enumerate(rows)}
        x_vr = np.ascontiguousarray(
            x[rows].reshape(RPC, TO, TI, D).reshape(B, TI, D)
        )
        g_vr = np.ascontiguousarray(
            g[rows].reshape(RPC, TO, TI, D).reshape(B, TI, D)
        )
        mmat = np.zeros((B, B), dtype=np.float32)
        for i, r in enumerate(rows):
            c_r = float(cvec[r])
            if c_r == 0.0:
                continue
            pl = loc[int(p_eff[r])]
            for to in range(TO):
                v = i * TO + to
                pv = pl * TO + to
                mmat[pv, v] += c_r
                mmat[v, v] -= c_r
        in_maps.append(
            {
                "x_vr": x_vr,
                "g_vr": g_vr,
                "m_mat": np.ascontiguousarray(mmat),
                "w_mat": wmat,
            }
        )

    res = run_bass_kernel_spmd(nc, in_maps, core_ids=list(range(N_CORES)))
    LAST_RESULT = res

    out = np.empty((B, T, D), dtype=np.float32)
    for c in range(N_CORES):
        o = res.results[c]["out_vr"].reshape(RPC, TO, TI, D).reshape(RPC, T, D)
        out[blocks[c]] = o
    return out


def _run_tshard(x, g, m, dm, p_eff):
    global LAST_RESULT
    from concourse.bass_utils import run_bass_kernel_spmd

    nc = _CACHE.get("tshard")
    if nc is None:
        nc = _build_tshard()
        _CACHE["tshard"] = nc
    dom_f = dm.astype(np.float32).reshape(B, 1)
    pmi = np.zeros((B, B), dtype=np.float32)
    pmi[p_eff, np.arange(B)] += 1.0
    pmi[np.arange(B), np.arange(B)] -= 1.0
    in_maps = []
    for c in range(N_CORES):
        sl = slice(c * T_LOC, (c + 1) * T_LOC)
        in_maps.append(
            {
                "x_sl": np.ascontiguousarray(x[:, sl, :]),
                "g_sl": np.ascontiguousarray(g[:, sl, :]),
                "m_in": m.reshape(B, 1),
                "dom_in": dom_f,
                "pmi_in": pmi,
            }
        )
    res = run_bass_kernel_spmd(nc, in_maps, core_ids=list(range(N_CORES)))
    LAST_RESULT = res
    out = np.empty((B, T, D), dtype=np.float32)
    for c in range(N_CORES):
        out[:, c * T_LOC : (c + 1) * T_LOC, :] = res.results[c]["out_sl"]
    return out


def _run_copy(x, g, m, dm, p_eff):
    global LAST_RESULT
    from concourse.bass_utils import run_bass_kernel_spmd

    nc = _CACHE.get("copy")
    if nc is None:
        nc = _build_copy()
        _CACHE["copy"] = nc
    dom_f = dm.astype(np.float32).reshape(B, 1)
    pmi = np.zeros((B, B), dtype=np.float32)
    in_maps = []
    for c in range(N_CORES):
        sl = slice(c * T_LOC, (c + 1) * T_LOC)
        in_maps.append(
            {
                "x_sl": np.ascontiguousarray(x[:, sl, :]),
                "g_sl": np.ascontiguousarray(g[:, sl, :]),
                "m_in": m.reshape(B, 1),
                "dom_in": dom_f,
                "pmi_in": pmi,
            }
        )
    res = run_bass_kernel_spmd(nc, in_maps, core_ids=list(range(N_CORES)))
    LAST_RESULT = res
    out = np.empty((B, T, D), dtype=np.float32)
    for c in range(N_CORES):
        out[:, c * T_LOC : (c + 1) * T_LOC, :] = res.results[c]["out_sl"]
    return out
